# revision 39
# baseline (speedup 1.0000x reference)
"""GCN + batch-attention kernel for Trainium2 (8 NeuronCores, SPMD).

Problem (nn_GCNResnet): for x [8192,3,10], A [3,3], W [10,10]:
    adj   = 0.25*(off_diag_ones + A)                    # normalized adjacency
    pooled= 0.5*(h0+h1),  h = adj @ (x @ W)             # -> [B,10]
    v     = softmax(pooled @ pooled.T) @ pooled         # -> [B,10]

Everything on device runs at HALF scale: the host folds an extra 0.5 into
Wc (incl. the ones column), so vn holds [pooled/2 | 1/2 | 0] and the S
matmul of two half-scale operands gives s/4 directly (numerator and
denominator both scale by 1/2, the final host divide cancels it). fp16
throughout (XBAR needs 2-byte; fp16's 10-bit mantissa keeps the end-to-end
error ~1e-3 vs bf16's ~2e-3).

Per core i (batch-sharded attention; input rolled by 1024*i rows so the
identical SPMD program always works on local rows 0:1024):
  - x2T loaded directly transposed from HBM via the XBAR DMA-transpose,
    one instruction + one SBUF tile per 1024-row supergroup; Wc rides in
    x2's free padding (cols 64:96 of rows 0:12) — a separate weight DMA
    would grab DMA engine 0 and chain the first x transpose behind it.
  - vn[g] [128,8,64] fp16 = natural-layout [pooled/2 | 1/2 | 0] rows via
    x2T @ Wc (PE -> pn PSUM -> one 96-wide copy). The transposed S-operands
    are NOT copied out of PSUM column-by-column (the old [12,512] copies
    burned ~12us of DVE); instead each group's vn is XBAR-transposed
    SBUF->SBUF on the DMA engines (4 tiles [128,2,64]->[128,128]; chunk
    u's lhsT lands at partition base 64*(u%2), the only bases the PE
    accepts besides 0/32, which is why vn pads d to 64).
  - the q-side rhs pT0h[h] [76,512] is produced once at startup by the PE
    writing the same pooled matmul at PSUM partition bases 0 AND 64 (the
    rhs must sit at the same base as the alternating lhsT), one copy each
    on the pre-exp-idle DVE/ACT.
  - flash attention, never materializing the [B,B] score matrix. The exp
    stream is the throughput floor; with the copies gone it is split
    ~34/30 between ACT (Exp activation, free affine scale=4) and DVE (a
    custom 8-stage polynomial op EXP_P4_ANT: E = T^4,
    T = 1 + y(1 + y(c0 + c1 y)), y = s/4; minimax c0,c1 over |s|<=3.65).
    Three 2-bank psum ring tiles (exp c reads tile c%3) keep the two
    engines consuming concurrently; the drain chunk 63 sits on ACT.
      S.T[kv c, q]/4 = trT[c]^T @ pT0h                     (PE, fp16)
      E [128,2,512] fp16 = exp-chunk                       (ACT or DVE)
      pvp[128 q-part, 8, 12] += E[:,h,qslice]^T @ vn[c]    (PE, fp16)
  - epilogue: one DVE copy pvp->SBUF, DMA the raw [num|den] rows out;
    the 8192x10 divide happens on the host.
PSUM: ring 3x2 banks + pvp 1 bank + pn small slot 1 bank = 8.
"""

import numpy as np

import concourse.bass as bass
import concourse.bacc as bacc
import concourse.mybir as mybir
import concourse.tile as tile
from concourse.bass_utils import run_bass_kernel_spmd

B = 8192
NCORES = 8
QL = B // NCORES          # 1024 local query rows
NF = 32                   # 30 feats + ones + zero pad (weight rows)
NFP = 128                 # host-padded feature columns for the XBAR transpose
D = 10
DV = 12                   # [pooled | 1 | 0]
DP = 64                   # vn d-padding so transposed chunks land 64 apart
NSG = 8                   # supergroups of 1024 batch rows
NKV = B // 128            # 64 kv chunks == 64 exp instructions

# minimax fit of (1 + y + c0 y^2 + c1 y^3)^4 ~ exp(4y) over |4y| <= 3.65
EXP_C0 = 0.52252056
EXP_C1 = 0.16330414
# 30 exp chunks on DVE (34 on ACT). The set comes from a makespan search
# over an event model of the steady state (engine serialization + the
# ring-reuse chain exp(c) -> S(c+3) -> exp(c+3), which binds at ring
# depth 3) with the measured per-chunk costs (ACT 1038ns / DVE 1192ns)
# and the DVE prologue copies at c = 1, 9, 17, 25, 33.
DVE_CHUNKS = frozenset((
    3, 4, 6, 8, 10, 12, 13, 17, 18, 20, 22, 24, 27, 29, 31,
    33, 35, 37, 39, 41, 43, 45, 47, 49, 51, 53, 55, 57, 60, 62))

f32 = mybir.dt.float32
fp16 = mybir.dt.float16
EXP = mybir.ActivationFunctionType.Exp

_NC = None
_EXP_OP = None


def _register_exp_op():
    """Register the cubic-then-squared-twice exp approximation as a custom
    DVE op via the dve_ops authoring surface (Spec -> per-NEFF uop table;
    no firmware change). Idempotent."""
    global _EXP_OP
    if _EXP_OP is not None:
        return _EXP_OP
    import concourse.dve_ops as dve_ops
    from concourse.dve_spec import Spec, Src0, C0, C1, One, sq, lower, _has_src1
    from concourse.dve_table_gen import dve_ver_for
    from concourse.dve_uop import DveOpSpec

    name = "EXP_P4_ANT"
    for op in dve_ops.OPS:
        if op.name == name:
            _EXP_OP = op
            return op

    def ref(in0, in1, s0, s1, imm2):
        y = in0.astype(np.float32)
        t = 1 + y * (1 + y * (s0 + y * s1))
        t2 = t * t
        return t2 * t2

    body = sq(sq(One + Src0 * (One + Src0 * (C0 + Src0 * C1))))
    op = dve_ops.DveOp(name, Spec(body=body, reference=ref),
                       subdim=False, uops_sha={})
    dve_ops.OPS.append(op)
    dve_ops._SUB_OPCODE_FOR_NAME[name] = (
        dve_ops._CUSTOM_DVE_ROW_BASE + len(dve_ops.OPS) - 1)
    dve_ops.CUSTOM_DVE_SPECS[name] = op.spec
    ver = dve_ver_for("TRN2")
    compiled = DveOpSpec(
        name=name, opcode=dve_ops.get_dve_sub_opcode(name),
        uops=lower(op.spec, ver=ver), rd1_en=_has_src1(op.spec))
    op.uops_sha[ver] = compiled.sha(ver)
    op.compile(ver)
    _EXP_OP = op
    return op


def _build():
    exp_op = _register_exp_op()
    nc = bacc.Bacc(trn_type="TRN2", target_bir_lowering=False)

    xr = nc.dram_tensor("xr", [B, NFP], fp16, kind="ExternalInput")
    v = nc.dram_tensor("v", [QL, DV], f32, kind="ExternalOutput")

    with tile.TileContext(nc) as tc:
        with (
            tc.tile_pool(name="const", bufs=1) as const,
            tc.tile_pool(name="bigp", bufs=1) as bigp,
            tc.tile_pool(name="epool", bufs=6) as epool,
            tc.tile_pool(name="ps", bufs=1, space="PSUM") as ps,
            tc.tile_pool(name="pssm", bufs=1, space="PSUM") as pssm,
        ):
            x2t0 = [bigp.tile([NFP, 512], fp16, tag=f"x2t0{h}",
                              name=f"x2t0{h}") for h in range(2)]
            x2t = [None] + [
                bigp.tile([NFP, QL], fp16, tag=f"x2t{g}", name=f"x2t{g}")
                for g in range(1, NSG)]
            wc_tile = const.tile([NF, DV], fp16, tag="wc")
            wc_sb = wc_tile[:, :]
            # q-side rhs, replicated at partition bases 0 and 64 so it can
            # match the lhsT base of every chunk.
            pT0h = [bigp.tile([76, 512], fp16, tag=f"pT0{h}", name=f"pT0{h}")
                    for h in range(2)]
            vn = [bigp.tile([128, NSG, DP], fp16, tag=f"vn{g}", name=f"vn{g}")
                  for g in range(NSG)]
            # trT[g][j]: chunks u=2j,2j+1 of group g as lhsT at bases 0/64
            trT = [None] + [
                [bigp.tile([128, 128], fp16, tag=f"tr{g}_{j}",
                           name=f"tr{g}_{j}") for j in range(4)]
                for g in range(1, NSG)]
            vout = bigp.tile([128, NSG, DV], f32, tag="vout")

            ringt = [ps.tile([128, 2, 512], f32, tag=f"ring{r}",
                             name=f"ring{r}") for r in range(3)]
            pvp = ps.tile([128, NSG, DV], f32, tag="pvp")   # 1 bank

            # PE warm-up with no DMA dependency (memset zeros, fp32 matmuls
            # keep PE busy from t=0 so the ramp model reaches full rate
            # before the first real matmul); the dummy exp pulls the
            # LoadActFuncSet off the first-chunk critical path.
            wz = const.tile([128, 128], f32, tag="wz")
            nc.vector.memset(wz[:, :], 0.0)
            # zero the vn pad columns once on the otherwise-idle Pool
            # engine so the XBAR transpose never reads uninitialized SBUF
            # (vn[0] is never transposed and needs no memset)
            for g in range(1, NSG):
                nc.gpsimd.memset(vn[g][:, :, :], 0.0)
            actwarm = const.tile([2, 2], f32, tag="actwarm")
            nc.scalar.activation(out=actwarm[:, :], in_=wz[0:2, 0:2], func=EXP)
            for w in range(9):
                nc.tensor.matmul(
                    ringt[2][:, 1, 64 * (w % 4):64 * (w % 4 + 1)],
                    wz[:, :], wz[:, 0:64],
                    start=True, stop=True,
                )

            # x supergroups land transposed straight from HBM on the SP
            # queue; Wc rides in x2's free padding (cols 64:96 of rows
            # 0:12) so the g0 transpose also delivers the weights — a
            # separate weight DMA would grab DMA engine 0 and chain the
            # first x transpose behind itself
            for h in range(2):
                nc.sync.dma_start(
                    out=x2t0[h][:, :],
                    in_=xr[512 * h:512 * (h + 1), :],
                    transpose=True,
                )
            for g in range(1, NSG):
                nc.sync.dma_start(
                    out=x2t[g][:, :],
                    in_=xr[QL * g:QL * (g + 1), :],
                    transpose=True,
                )
            nc.vector.tensor_copy(wc_sb, x2t0[0][64:96, 0:DV])

            # ---- startup: pT0h via PE writing both partition bases.
            # Deps are tile-granular and engine semaphores are counting,
            # so each staging matmul gets its OWN ring tile and consumers
            # are emitted so their sem thresholds stay minimal: pp h0 in
            # ring0, pp h1 in ring1 (their base-0 copies gate S(0)); the
            # base-64 clones stage in ring2 between S(1) and S(2) — only
            # S(2) WARs their copies, and they are not needed before S(9).
            nc.tensor.matmul(ringt[0][0:DV, 0, :], wc_sb,
                             x2t0[0][0:NF, :], start=True, stop=True)
            nc.tensor.matmul(ringt[1][0:DV, 0, :], wc_sb,
                             x2t0[1][0:NF, :], start=True, stop=True)
            nc.vector.tensor_copy(pT0h[0][0:DV, :], ringt[0][0:DV, 0, :])
            nc.scalar.copy(pT0h[1][0:DV, :], ringt[1][0:DV, 0, :])

            def pro_vnat(g):
                """natural-layout [pooled/2 | 1/2 | 0] rows -> pn PSUM."""
                pn = pssm.tile([128, NSG * DV], f32, tag="sm", name="pn")
                for u in range(NSG):
                    src = x2t0[u // 4][0:NF, 128 * (u % 4):128 * (u % 4 + 1)] \
                        if g == 0 else x2t[g][0:NF, 128 * u:128 * (u + 1)]
                    nc.tensor.matmul(
                        pn[:, DV * u:DV * (u + 1)], src, wc_sb,
                        start=(u == 0), stop=(u == NSG - 1),
                    )
                return pn

            def vn_copy(g, pn, eng):
                eng(vn[g][:, :, 0:DV],
                    pn[:, :].rearrange("p (u d) -> p u d", u=NSG))

            def emit_tr(g, j):
                """chunks u=2j,2j+1 of group g -> trT[g][j] (SP hwdge
                queue, after all the x loads: its wait for the vn copy
                head-blocks only later transposes and the final out-DMA,
                never the exp engines)."""
                nc.sync.dma_start(
                    out=trT[g][j][:, :],
                    in_=vn[g][:, 2 * j:2 * j + 2, :],
                    transpose=True,
                )

            def pt_cols(c):
                if c < 8:
                    return pT0h[c // 4][0:D, 128 * (c % 4):128 * (c % 4 + 1)], 0
                u = c % 8
                b = 64 * (u % 2)
                return trT[c // 8][u // 2][b:b + D, :], b

            def emit_s(c):
                """S/4 for chunk c -> ring tile c%3 (slot h = q-half)."""
                lhsT, b = pt_cols(c)
                for h in range(2):
                    nc.tensor.matmul(
                        ringt[c % 3][:, h, :],
                        lhsT,
                        pT0h[h][b:b + D, :],
                        start=True, stop=True,
                    )

            def emit_exp(c):
                """E[c] = exp(4 * ring tile c%3) on ACT, or the custom DVE
                polynomial (chunks in DVE_CHUNKS)."""
                et = epool.tile([128, 2, 512], fp16, tag="E", name="et")
                if c in DVE_CHUNKS:
                    nc.vector._custom_dve(
                        exp_op, out=et[:, :, :], in0=ringt[c % 3][:, :, :],
                        s0=EXP_C0, s1=EXP_C1)
                else:
                    nc.scalar.activation(
                        out=et[:, :, :], in_=ringt[c % 3][:, :, :],
                        func=EXP, scale=4.0)
                return et

            def emit_pv(c, et):
                """pvp[:, 4h+j, :] += E[:, h, 128j:...]^T @ vn[chunk c].

                The whole pvp bank is ONE psum zero-region: only the very
                first matmul carries start, only the very last carries
                stop."""
                for h in range(2):
                    for j in range(4):
                        nc.tensor.matmul(
                            pvp[:, 4 * h + j, :],
                            et[:, h, 128 * j:128 * (j + 1)],
                            vn[c // 8][:, c % 8, 0:DV],
                            start=(c == 0 and h == 0 and j == 0),
                            stop=(c == NKV - 1 and h == 1 and j == 3),
                        )

            # group-0/1 prologue + first three chunks' S up front; later
            # groups' prologue pieces are spread across the loop so the PE
            # stays fed while the exp streams run.
            emit_s(0)
            emit_s(1)
            # base-64 clones (see startup comment above)
            for h in range(2):
                nc.tensor.matmul(ringt[2][64:64 + DV, h, :], wc_sb,
                                 x2t0[h][0:NF, :], start=True, stop=True)
            for h in range(2):
                nc.vector.tensor_copy(pT0h[h][64:64 + DV, :],
                                      ringt[2][64:64 + DV, h, :])
            emit_s(2)
            # groups 0-2 prologue up front: the copies/transposes fill the
            # DVE idle window before its first exp chunk (c=3)
            for g in range(3):
                png = pro_vnat(g)
                vn_copy(g, png, nc.vector.tensor_copy)
                if g >= 1:
                    for j in range(4):
                        emit_tr(g, j)

            et_prev = None
            pns = {}
            for c in range(NKV):
                et_cur = emit_exp(c)
                if c + 3 < NKV:
                    emit_s(c + 3)
                if et_prev is not None:
                    emit_pv(c - 1, et_prev)
                et_prev = et_cur
                g_next = c // 8 + 3
                if g_next < NSG:
                    m = c % 8
                    if m == 0:
                        pns[g_next] = pro_vnat(g_next)
                    elif m == 1:
                        vn_copy(g_next, pns[g_next], nc.vector.tensor_copy)
                        for j in range(4):
                            emit_tr(g_next, j)
            emit_pv(NKV - 1, et_prev)

            # ---- epilogue: raw [num|den] rows out; host divides
            nc.vector.tensor_copy(vout[:, :, :], pvp[:, :, :])
            dst = bass.AP(v, 0, [[DV, 128], [128 * DV, NSG], [1, DV]])
            nc.sync.dma_start(out=dst, in_=vout[:, :, :])

    nc.finalize()
    return nc


def _get_nc():
    global _NC
    if _NC is None:
        _NC = _build()
    return _NC


def _host_fold(A, W):
    """Fold adjacency normalization + node pooling + a global 1/2 into one
    [32,12] weight. Column 10 holds the half-scale ones column (so the
    denominator scales with the numerator and the host divide cancels the
    1/2); rows 31+/col 11 are zero padding."""
    A = np.asarray(A, np.float32)
    W = np.asarray(W, np.float32)
    off = np.ones((3, 3), np.float32) - np.eye(3, dtype=np.float32)
    a = off + A
    d = 0.5 * np.eye(3, dtype=np.float32)
    adj = (d @ a @ d).astype(np.float32)
    c = (0.25 * (adj[0, :] + adj[1, :])).astype(np.float32)  # 0.5 pool * 0.5
    wcm = np.zeros((NF, DV), np.float32)
    wcm[0:30, 0:D] = np.einsum("n,fo->nfo", c, W).reshape(30, D)
    wcm[30, D] = 0.5
    return wcm.astype(np.float16)


def _host_x2(x):
    x2 = np.zeros((B, NFP), np.float16)
    x2[:, 0:30] = np.asarray(x, np.float32).reshape(B, 30).astype(np.float16)
    x2[:, 30] = 1.0
    return x2


def _core_xr(x2, wcm, i):
    """Core i's input: rolled x2 with Wc.T embedded in the free padding
    (cols 64:96 of rows 0:12) so the g0 transpose also delivers the
    weights."""
    xc = np.roll(x2, -QL * i, axis=0)
    xc[0:DV, 64:96] = wcm.T
    return xc


def _host_finish(raw):
    """raw [QL, 12] per core -> v rows: numerator/denominator."""
    raw = np.asarray(raw, np.float32)
    return raw[:, 0:D] / raw[:, D:D + 1]


def kernel(x, A, W):
    wcm = _host_fold(A, W)
    x2 = _host_x2(x)

    nc = _get_nc()
    in_maps = [{"xr": _core_xr(x2, wcm, i)} for i in range(NCORES)]
    res = run_bass_kernel_spmd(nc, in_maps, core_ids=list(range(NCORES)))
    return np.concatenate(
        [_host_finish(res.results[i]["v"]) for i in range(NCORES)], axis=0)
